# revision 30
# baseline (speedup 1.0000x reference)
"""Multi-head attention (B=8, S=1024, D=1024, H=16) on 8 trn2 NeuronCores.

Strategy: batch-parallel (1 batch per core), zero collectives.

Per-core pipeline (all on-chip, no DRAM round-trips):
  - kT (f32r) and v (paired bf16 v_aug layout) are SBUF-resident; per head
    pair: Q-projection, scores (f32r, 64-row contraction at partition offset
    j*64), exp on ACT (bf16 out), attn@V with an augmented-ones column giving
    the softmax denominator for free.
  - v_aug pair layout [v_even(64) | ones_e | ones_o | zeros(31) | v_odd(64)]
    lands the odd head's A@V on PSUM partitions 64:128 so both catT halves
    are written by lane-aligned DVE ops; 1/den (fast-approx reciprocal) is
    broadcast across partitions with a rank-1 PE matmul whose emission is
    deferred ~half a head so the PE queue never waits on the chain.
  - The PE clock gate (HAM) downclocks 2.4->1.2 GHz on any idle window, so
    K-projection tiles, the next pair's Q-projection, the second V half and
    partial O-projection passes are interleaved into the attention stream
    as filler to keep the array saturated.
"""

import sys

if "/opt/trn_rl_repo" not in sys.path:
    sys.path.insert(0, "/opt/trn_rl_repo")

import numpy as np

B, S, D, H = 8, 1024, 1024, 16
Dh = D // H  # 64
P = 128
NT = 8  # number of 128-wide chunks in 1024
SH = 512
VW = 161  # v_aug per-(tt,pair) width: 64 + 1 + 1 + 31 + 64

_CACHE = {}


def _bf16():
    import ml_dtypes

    return ml_dtypes.bfloat16


def _prep_x(x):
    # x [S, D] -> [128, 8192]; out[p, k*1024 + s] = x[s, k*128+p]
    return np.ascontiguousarray(
        x.reshape(S, NT, P).transpose(2, 1, 0).reshape(P, NT * S)
    ).astype(_bf16())


def _prep_w(Wcat):
    # W [out 1024, in 1024] -> [8, 128, 1024]; out[et, p, k*128+oc] = W[et*128+oc, k*128+p]
    return np.ascontiguousarray(
        Wcat.reshape(NT, P, NT, P).transpose(0, 3, 2, 1).reshape(NT, P, NT * P)
    ).astype(_bf16())


def _prep_wv(Wvcat):
    # [128, 8192]; out[p, k*1024 + e] = Wv_cat[e, k*128+p]
    return np.ascontiguousarray(
        Wvcat.T.reshape(NT, P, D).transpose(1, 0, 2).reshape(P, NT * D)
    ).astype(_bf16())


def _prep_bias(b):
    # [1024] -> [128, 8]; out[p, i] = b[i*128+p]
    return np.ascontiguousarray(b.reshape(NT, P).T)


def _make_in_maps(query, key, value, Wq, bq, Wk, bk, Wv, bv, Wo, bo):
    query = np.asarray(query, np.float32)
    key = np.asarray(key, np.float32)
    value = np.asarray(value, np.float32)
    Wq_c = np.asarray(Wq, np.float32).reshape(D, D)
    Wk_c = np.asarray(Wk, np.float32).reshape(D, D)
    Wv_c = np.asarray(Wv, np.float32).reshape(D, D)
    Wo_c = np.asarray(Wo, np.float32)
    bq_c = np.asarray(bq, np.float32).reshape(D)
    bk_c = np.asarray(bk, np.float32).reshape(D)
    bv_c = np.asarray(bv, np.float32).reshape(D)
    bo_c = np.asarray(bo, np.float32)

    shared = {
        "wq": _prep_w(Wq_c),
        "wk": _prep_w(Wk_c),
        "wv": _prep_wv(Wv_c),
        "wo": _prep_w(Wo_c),
        "bqd": _prep_bias(bq_c),
        "bkd": _prep_bias(bk_c),
        # attn rows sum to 1, so  attn @ (v + bv) = attn @ v + bv, and bv then
        # flows through the output projection as an extra bias Wo @ bv.
        "bod": _prep_bias(bo_c + Wo_c @ bv_c),
    }
    in_maps = []
    for b in range(B):
        m = dict(shared)
        m["xq"] = _prep_x(query[b])
        m["xk"] = _prep_x(key[b])
        m["xv"] = _prep_x(value[b])
        in_maps.append(m)
    return in_maps


def _build():
    import concourse.mybir as mybir
    import concourse.tile as tile
    from concourse import bacc

    dt = mybir.dt
    f32 = dt.float32
    f32r = dt.float32r
    bf16 = dt.bfloat16
    AF = mybir.ActivationFunctionType
    ALU = mybir.AluOpType

    nc = bacc.Bacc(None, target_bir_lowering=False)

    with tile.TileContext(nc) as tc:
        with (
            tc.tile_pool(name="dram", bufs=1, space="DRAM") as dram,
            tc.tile_pool(name="consts", bufs=1) as consts,
            tc.tile_pool(name="xa_p", bufs=1) as xa_p,
            tc.tile_pool(name="xv_p", bufs=1) as xv_p,
            tc.tile_pool(name="xq_p", bufs=1) as xq_p,
            tc.tile_pool(name="wst_p", bufs=3) as wst_p,
            tc.tile_pool(name="wv_p", bufs=1) as wv_p,
            tc.tile_pool(name="kt_p", bufs=1) as kt_p,
            tc.tile_pool(name="vaug_p", bufs=1) as vaug_p,
            tc.tile_pool(name="cat_p", bufs=1) as cat_p,
            tc.tile_pool(name="qp_p", bufs=2) as qp_p,
            tc.tile_pool(name="ex_p", bufs=4) as ex_p,
            tc.tile_pool(name="aj_p", bufs=2) as aj_p,
            tc.tile_pool(name="rd_p", bufs=2) as rd_p,
            tc.tile_pool(name="st_p", bufs=2) as st_p,
            tc.tile_pool(name="ps", bufs=3, space="PSUM") as ps_p,
        ):
            # ---- DRAM I/O ----
            xq = dram.tile([P, NT * S], bf16, kind="ExternalInput", name="xq", uniquify=False)
            xk = dram.tile([P, NT * S], bf16, kind="ExternalInput", name="xk", uniquify=False)
            xv = dram.tile([P, NT * S], bf16, kind="ExternalInput", name="xv", uniquify=False)
            wq = dram.tile([NT, P, D], bf16, kind="ExternalInput", name="wq", uniquify=False)
            wk = dram.tile([NT, P, D], bf16, kind="ExternalInput", name="wk", uniquify=False)
            wv = dram.tile([P, NT * D], bf16, kind="ExternalInput", name="wv", uniquify=False)
            wo = dram.tile([NT, P, D], bf16, kind="ExternalInput", name="wo", uniquify=False)
            bqd = dram.tile([P, NT], f32, kind="ExternalInput", name="bqd", uniquify=False)
            bkd = dram.tile([P, NT], f32, kind="ExternalInput", name="bkd", uniquify=False)
            bod = dram.tile([P, NT], f32, kind="ExternalInput", name="bod", uniquify=False)
            outT = dram.tile([NT, P, S], f32, kind="ExternalOutput", name="outT", uniquify=False)

            # ---- startup DMAs: xk first (K-projection runs first) ----
            xk_sb = xa_p.tile([P, NT * S], bf16, name="xk_sb", tag="xa")
            nc.sync.dma_start(xk_sb[:], xk[:])

            bq_sb = consts.tile([P, NT], f32, name="bq_sb")
            bk_sb = consts.tile([P, NT], f32, name="bk_sb")
            bo_sb = consts.tile([P, NT], f32, name="bo_sb")
            nc.sync.dma_start(bk_sb[:], bkd[:])
            nc.sync.dma_start(bq_sb[:], bqd[:])
            nc.sync.dma_start(bo_sb[:], bod[:])

            kT_sb = kt_p.tile([P, NT * S], f32r, name="kT_sb")
            v_aug = vaug_p.tile([P, NT, NT, VW], bf16, name="v_aug")
            catT = cat_p.tile([P, NT, S], bf16, name="catT")

            # ISA memset only writes plain f32 -> stage in f32 and cast-copy
            # the ones columns / zero filler / f32r ones row.
            ones_sb = consts.tile([1, P], f32r, name="ones_sb")
            zf = consts.tile([P, NT * 2 * 31], f32, name="zf")
            nc.vector.memset(zf[:], 0.0)
            for g in range(4):
                nc.vector.tensor_copy(
                    v_aug[:, :, 2 * g : 2 * g + 2, Dh + 2 : 97],
                    zf[:].rearrange("p (a b c) -> p a b c", b=2, c=31),
                )
            nc.vector.memset(zf[:, 0 : NT * NT * 2], 1.0)
            nc.vector.tensor_copy(
                v_aug[:, :, :, Dh : Dh + 2],
                zf[:, 0 : NT * NT * 2].rearrange("p (a b c) -> p a b c", b=NT, c=2),
            )
            with nc.allow_low_precision(reason="f32r is bit-identical to f32"):
                nc.vector.tensor_copy(ones_sb[:], zf[0:1, 0:P])

            # ---- K-projection helpers (et 0..1 up front, 2..7 interleaved) ----
            def k_proj_start(et):
                w = wst_p.tile([P, D], bf16, name="wkt", tag="w")
                nc.sync.dma_start(w[:], wk[et])
                return w

            def k_proj_chunk(hold, lo, hi):
                w, ps = hold
                for k in range(lo, hi):
                    for sh in range(2):
                        nc.tensor.matmul(
                            ps[:, sh * SH : (sh + 1) * SH],
                            w[:, k * P : (k + 1) * P],
                            xk_sb[:, k * S + sh * SH : k * S + (sh + 1) * SH],
                            start=(k == 0),
                            stop=(k == NT - 1),
                        )

            def k_proj_finish(et, hold):
                nc.vector.tensor_scalar_add(
                    kT_sb[:, et * S : (et + 1) * S], hold[1][:], bk_sb[:, et : et + 1]
                )

            for et in range(2):
                w = k_proj_start(et)
                ps = ps_p.tile([P, S], f32, name="pp", tag="sc", bufs=2)
                k_proj_chunk((w, ps), 0, NT)
                k_proj_finish(et, (w, ps))

            # ---- V inputs ----
            xv_sb = xv_p.tile([P, NT * S], bf16, name="xv_sb", tag="xv")
            nc.sync.dma_start(xv_sb[:], xv[:])
            wv_sb = wv_p.tile([P, NT * D], bf16, name="wv_sb", tag="wv")
            nc.sync.dma_start(wv_sb[:], wv[:])

            # ---- V-projection, one e-half at a time ----
            def v_proj_tt(eh, tt, tag="hold", bufs=1):
                ps = ps_p.tile([P, SH], f32, name="pv", tag=tag, bufs=bufs)
                for k in range(NT):
                    nc.tensor.matmul(
                        ps[:],
                        xv_sb[:, k * S + tt * P : k * S + (tt + 1) * P],
                        wv_sb[:, k * D + eh * SH : k * D + (eh + 1) * SH],
                        start=(k == 0),
                        stop=(k == NT - 1),
                    )
                prs = ps[:].rearrange("p (r j c) -> p r j c", j=2, c=Dh)
                pr0 = eh * 4
                nc.vector.tensor_copy(
                    v_aug[:, tt, pr0 : pr0 + 4, 0:Dh], prs[:, :, 0, :]
                )
                nc.vector.tensor_copy(
                    v_aug[:, tt, pr0 : pr0 + 4, 97 : 97 + Dh], prs[:, :, 1, :]
                )

            for tt in range(5):
                v_proj_tt(0, tt, tag="sc", bufs=2)

            # ---- Q input ----
            xq_sb = xq_p.tile([P, NT * S], bf16, name="xq_sb", tag="xq")
            nc.sync.dma_start(xq_sb[:], xq[:])

            def q_proj(qtile, lo, hi):
                w, ps = qtile
                for k in range(lo, hi):
                    for sh in range(2):
                        nc.tensor.matmul(
                            ps[:, sh * SH : (sh + 1) * SH],
                            w[:, k * P : (k + 1) * P],
                            xq_sb[:, k * S + sh * SH : k * S + (sh + 1) * SH],
                            start=(k == 0),
                            stop=(k == NT - 1),
                        )

            wqt = wst_p.tile([P, D], bf16, name="wqt", tag="w")
            nc.sync.dma_start(wqt[:], wq[0])
            qps = ps_p.tile([P, S], f32, name="pq", tag="sc", bufs=2)
            q_proj((wqt, qps), 0, NT)
            qp = qp_p.tile([P, S], f32r, name="qp", tag="qp")
            nc.vector.tensor_scalar_add(qp[:], qps[:], bq_sb[:, 0:1])

            # ---- partial O-projection (pass A: et 0..3), used as PE filler ----
            # oacc reuses xk's pool slot (xk is dead once the last K-proj
            # chunk has run, before the first pass-A tile is emitted)
            oacc_cell = []

            def o_pass(ft, elo, ehi):
                if not oacc_cell:
                    oacc_cell.append(
                        xa_p.tile([P, NT, S], f32, name="oacc", tag="xa")
                    )
                oacc = oacc_cell[0]
                w = wst_p.tile([P, D], bf16, name="wot", tag="w")
                nc.sync.dma_start(w[:], wo[ft])
                ps = ps_p.tile([P, S], f32, name="po", tag="hold", bufs=1)
                for et in range(elo, ehi):
                    for sh in range(2):
                        nc.tensor.matmul(
                            ps[:, sh * SH : (sh + 1) * SH],
                            w[:, et * P : (et + 1) * P],
                            catT[:, et, sh * SH : (sh + 1) * SH],
                            start=(et == elo),
                            stop=(et == ehi - 1),
                        )
                if elo == 0:
                    nc.vector.tensor_copy(oacc[:, ft, :], ps[:])
                else:
                    nc.vector.tensor_add(oacc[:, ft, :], oacc[:, ft, :], ps[:])

            # ---- deferred head finalize ----
            pending = []

            def emit_pending():
                while pending:
                    pending.pop(0)()

            def finalize_evac(pr, j, av):
                dl = Dh if j == 0 else 32  # den lane in av
                e0 = j * Dh
                aj = aj_p.tile([P, S], f32, name="aj", tag="aj")
                if j == 0:
                    nc.vector.tensor_copy(aj[0 : Dh + 1, :], av[0 : Dh + 1, :])
                else:
                    # engine APs may only start at partition 0/32/64/96
                    # (<=32 from 32): copy all 128 rows from 0 instead
                    nc.vector.tensor_copy(aj[:, :], av[:, :])
                rdt = rd_p.tile([1, 2 * S], f32, name="rdt", tag="rd", bufs=1)
                rdr = rd_p.tile([1, S], f32r, name="rdr", tag="rdr", bufs=1)
                nc.sync.dma_start(rdt[0:1, 0:S], aj[dl : dl + 1, :])
                nc.vector.reciprocal_approx_fast(
                    out=rdt[0:1, S : 2 * S], in_=rdt[0:1, 0:S]
                )
                with nc.allow_low_precision(reason="round 1/den to f32r for PE"):
                    nc.vector.tensor_copy(rdr[0:1, :], rdt[0:1, S : 2 * S])

                def emit_bc():
                    bc = ps_p.tile([P, S], f32, name="bc", tag="hold", bufs=1)
                    for sh in range(2):
                        nc.tensor.matmul(
                            bc[:, sh * SH : (sh + 1) * SH],
                            ones_sb[:, :],
                            rdr[0:1, sh * SH : (sh + 1) * SH],
                        )
                    nc.vector.tensor_mul(
                        catT[e0 : e0 + Dh, pr, :],
                        aj[e0 : e0 + Dh, :],
                        bc[e0 : e0 + Dh, :],
                    )

                pending.append(emit_bc)

            # ---- fused attention, one head pair (pr) at a time ----
            # Filler placement per pr (PE stays saturated, HAM stays at 2.4GHz):
            #   j0 tt3: previous head's deferred bc; tt4-7: K-proj et=pr+2
            #           (or an O pass-A tile for pr 6/7)
            #   j1 tt1/tt3: next pair's q-projection; tt5: this j0's bc;
            #           tt5/tt7: V-proj second half (pr<4) or O pass-A (pr 6/7)
            for pr in range(NT):
                if pr + 1 < NT:
                    wqt_n = wst_p.tile([P, D], bf16, name="wqt", tag="w")
                    nc.sync.dma_start(wqt_n[:], wq[pr + 1])
                k_ets = (
                    [pr + 2] if pr <= 2 else
                    ([5, 6] if pr == 3 else ([7] if pr == 4 else []))
                )
                k_ws = [k_proj_start(e) for e in k_ets]

                qp_next = None
                for j in range(2):
                    e0 = j * Dh
                    av = ps_p.tile([P, S], f32, name="av", tag="av", bufs=1)
                    # v_aug stationary view for this head:
                    #  j=0: [v_even(64) | ones_e] -> av rows 0:64, den row 64
                    #  j=1: offset 33 -> ones_o at col 32, v_odd at cols 64:128
                    #       -> den row 32, av rows 64:128
                    if j == 0:
                        vst = v_aug[:, :, pr, 0 : Dh + 1]
                        avw = av[0 : Dh + 1, :]
                    else:
                        vst = v_aug[:, :, pr, 33 : 33 + P]
                        avw = av[:]
                    for tt in range(NT):
                        sc = ps_p.tile([P, S], f32, name="sc", tag="sc", bufs=2)
                        for sh in range(2):
                            nc.tensor.matmul(
                                sc[:, sh * SH : (sh + 1) * SH],
                                kT_sb[e0 : e0 + Dh, pr * S + tt * P : pr * S + (tt + 1) * P],
                                qp[e0 : e0 + Dh, sh * SH : (sh + 1) * SH],
                            )
                        ex = ex_p.tile([P, S], bf16, name="ex", tag="ex")
                        nc.scalar.activation(ex[:], sc[:], AF.Exp, scale=0.125)
                        for sh in range(2):
                            nc.tensor.matmul(
                                avw[:, sh * SH : (sh + 1) * SH],
                                vst[:, tt, :],
                                ex[:, sh * SH : (sh + 1) * SH],
                                start=(tt == 0),
                                stop=(tt == NT - 1),
                            )
                        if j == 0:
                            if pr == 0 and tt in (1, 3, 5):
                                v_proj_tt(0, 5 + (tt - 1) // 2)
                            if tt == 3:
                                emit_pending()  # pr-1 j1's bc
                            if tt in (4, 6):
                                ki = 0 if tt == 4 else 1
                                if ki < len(k_ets):
                                    k_ps = ps_p.tile([P, S], f32, name="pp", tag="hold", bufs=1)
                                    k_proj_chunk((k_ws[ki], k_ps), 0, NT)
                                    k_proj_finish(k_ets[ki], (k_ws[ki], k_ps))
                            if pr == 4 and tt == 6:
                                o_pass(0, 0, 3)
                            if pr == 5 and tt in (4, 6):
                                o_pass(3 if tt == 4 else 4, 0, 3)
                            if pr == 6 and tt in (4, 6):
                                o_pass(7 if tt == 4 else None, 0, 3) if tt == 4 else o_pass(0, 3, 5)
                            if pr == 7 and tt in (4, 6):
                                o_pass(3 if tt == 4 else 4, 3, 5)
                        else:
                            if pr + 1 < NT and tt == 2:
                                qps_n = ps_p.tile([P, S], f32, name="pq", tag="hold", bufs=1)
                                q_proj((wqt_n, qps_n), 0, NT)
                                qp_next = qp_p.tile([P, S], f32r, name="qp", tag="qp")
                                nc.vector.tensor_scalar_add(
                                    qp_next[:], qps_n[:], bq_sb[:, pr + 1 : pr + 2]
                                )
                            if tt == 5:
                                emit_pending()  # this pr j0's bc
                            if pr < 4 and tt in (4, 6):
                                v_proj_tt(1, 2 * pr + (0 if tt == 4 else 1))
                            if pr == 4 and tt in (4, 6):
                                o_pass(1 if tt == 4 else 2, 0, 3)
                            if pr == 5 and tt in (4, 6):
                                o_pass(5 if tt == 4 else 6, 0, 3)
                            if pr == 6 and tt in (4, 6):
                                o_pass(1 if tt == 4 else 2, 3, 5)
                            if pr == 7 and tt in (4, 6):
                                for f2 in ((5, 6) if tt == 4 else (7,)):
                                    o_pass(f2, 3, 5)

                    finalize_evac(pr, j, av)
                if qp_next is not None:
                    qp = qp_next

            # ---------------- output projection, pass B (et 4..7) ----------------
            for ft in range(NT):
                w = wst_p.tile([P, D], bf16, name="wot", tag="w")
                nc.sync.dma_start(w[:], wo[ft])
                ps = ps_p.tile([P, S], f32, name="po", tag="sc", bufs=2)
                for et in range(5, NT):
                    # pr7-j1's deferred bc: cover its reciprocal with the
                    # first accumulation steps (catT[:, 4:7] is ready)
                    if ft == 0 and et == NT - 1:
                        emit_pending()
                    for sh in range(2):
                        nc.tensor.matmul(
                            ps[:, sh * SH : (sh + 1) * SH],
                            w[:, et * P : (et + 1) * P],
                            catT[:, et, sh * SH : (sh + 1) * SH],
                            start=(et == 5),
                            stop=(et == NT - 1),
                        )
                st = st_p.tile([P, S], f32, name="so", tag="st")
                nc.vector.scalar_tensor_tensor(
                    st[:], ps[:], bo_sb[:, ft : ft + 1], oacc_cell[0][:, ft, :],
                    ALU.add, ALU.add,
                )
                nc.sync.dma_start(outT[ft], st[:])

    nc.compile()
    return nc


def kernel(query, key, value, mask, Wq, bq, Wk, bk, Wv, bv, Wo, bo):
    from concourse.bass_utils import run_bass_kernel_spmd

    if "nc" not in _CACHE:
        _CACHE["nc"] = _build()
    nc = _CACHE["nc"]

    in_maps = _make_in_maps(query, key, value, Wq, bq, Wk, bk, Wv, bv, Wo, bo)
    res = run_bass_kernel_spmd(nc, in_maps, core_ids=list(range(B)))
    out = np.empty((B, S, D), np.float32)
    for b in range(B):
        out[b] = res.results[b]["outT"].reshape(D, S).T
    return out


# revision 31
# speedup vs baseline: 1.0343x; 1.0343x over previous
"""Multi-head attention (B=8, S=1024, D=1024, H=16) on 8 trn2 NeuronCores.

Strategy: batch-parallel (1 batch per core), zero collectives.

Per-core pipeline (all on-chip, no DRAM round-trips):
  - kT (f32r) and v (paired bf16 v_aug layout) are SBUF-resident; per head
    pair: Q-projection, scores (f32r, 64-row contraction at partition offset
    j*64), exp on ACT (bf16 out), attn@V with an augmented-ones column giving
    the softmax denominator for free.
  - v_aug pair layout [v_even(64) | ones_e | ones_o | zeros(31) | v_odd(64)]
    lands the odd head's A@V on PSUM partitions 64:128 so both catT halves
    are written by lane-aligned DVE ops; 1/den (fast-approx reciprocal) is
    broadcast across partitions with a rank-1 PE matmul whose emission is
    deferred ~half a head so the PE queue never waits on the chain.
  - The PE clock gate (HAM) downclocks 2.4->1.2 GHz on any idle window, so
    K-projection tiles, the next pair's Q-projection, the second V half and
    partial O-projection passes are interleaved into the attention stream
    as filler to keep the array saturated.
"""

import sys

if "/opt/trn_rl_repo" not in sys.path:
    sys.path.insert(0, "/opt/trn_rl_repo")

import numpy as np

B, S, D, H = 8, 1024, 1024, 16
Dh = D // H  # 64
P = 128
NT = 8  # number of 128-wide chunks in 1024
SH = 512
VW = 161  # v_aug per-(tt,pair) width: 64 + 1 + 1 + 31 + 64

_CACHE = {}


def _bf16():
    import ml_dtypes

    return ml_dtypes.bfloat16


def _prep_x(x):
    # x [S, D] -> [128, 8192]; out[p, k*1024 + s] = x[s, k*128+p]
    return np.ascontiguousarray(
        x.reshape(S, NT, P).transpose(2, 1, 0).reshape(P, NT * S)
    ).astype(_bf16())


def _prep_w(Wcat):
    # W [out 1024, in 1024] -> [8, 128, 1024]; out[et, p, k*128+oc] = W[et*128+oc, k*128+p]
    return np.ascontiguousarray(
        Wcat.reshape(NT, P, NT, P).transpose(0, 3, 2, 1).reshape(NT, P, NT * P)
    ).astype(_bf16())


def _prep_wv(Wvcat):
    # [128, 8192]; out[p, k*1024 + e] = Wv_cat[e, k*128+p]
    return np.ascontiguousarray(
        Wvcat.T.reshape(NT, P, D).transpose(1, 0, 2).reshape(P, NT * D)
    ).astype(_bf16())


def _prep_bias(b):
    # [1024] -> [128, 8]; out[p, i] = b[i*128+p]
    return np.ascontiguousarray(b.reshape(NT, P).T)


def _make_in_maps(query, key, value, Wq, bq, Wk, bk, Wv, bv, Wo, bo):
    query = np.asarray(query, np.float32)
    key = np.asarray(key, np.float32)
    value = np.asarray(value, np.float32)
    Wq_c = np.asarray(Wq, np.float32).reshape(D, D)
    Wk_c = np.asarray(Wk, np.float32).reshape(D, D)
    Wv_c = np.asarray(Wv, np.float32).reshape(D, D)
    Wo_c = np.asarray(Wo, np.float32)
    bq_c = np.asarray(bq, np.float32).reshape(D)
    bk_c = np.asarray(bk, np.float32).reshape(D)
    bv_c = np.asarray(bv, np.float32).reshape(D)
    bo_c = np.asarray(bo, np.float32)

    shared = {
        "wq": _prep_w(Wq_c),
        "wk": _prep_w(Wk_c),
        "wv": _prep_wv(Wv_c),
        "wo": _prep_w(Wo_c),
        "bqd": _prep_bias(bq_c),
        "bkd": _prep_bias(bk_c),
        # attn rows sum to 1, so  attn @ (v + bv) = attn @ v + bv, and bv then
        # flows through the output projection as an extra bias Wo @ bv.
        "bod": _prep_bias(bo_c + Wo_c @ bv_c),
    }
    in_maps = []
    for b in range(B):
        m = dict(shared)
        m["xq"] = _prep_x(query[b])
        m["xk"] = _prep_x(key[b])
        m["xv"] = _prep_x(value[b])
        in_maps.append(m)
    return in_maps


def _build():
    import concourse.mybir as mybir
    import concourse.tile as tile
    from concourse import bacc

    dt = mybir.dt
    f32 = dt.float32
    f32r = dt.float32r
    bf16 = dt.bfloat16
    AF = mybir.ActivationFunctionType
    ALU = mybir.AluOpType

    nc = bacc.Bacc(None, target_bir_lowering=False)

    with tile.TileContext(nc) as tc:
        with (
            tc.tile_pool(name="dram", bufs=1, space="DRAM") as dram,
            tc.tile_pool(name="consts", bufs=1) as consts,
            tc.tile_pool(name="xa_p", bufs=1) as xa_p,
            tc.tile_pool(name="xv_p", bufs=1) as xv_p,
            tc.tile_pool(name="xq_p", bufs=1) as xq_p,
            tc.tile_pool(name="wst_p", bufs=3) as wst_p,
            tc.tile_pool(name="wv_p", bufs=1) as wv_p,
            tc.tile_pool(name="kt_p", bufs=1) as kt_p,
            tc.tile_pool(name="vaug_p", bufs=1) as vaug_p,
            tc.tile_pool(name="cat_p", bufs=1) as cat_p,
            tc.tile_pool(name="qp_p", bufs=2) as qp_p,
            tc.tile_pool(name="ex_p", bufs=4) as ex_p,
            tc.tile_pool(name="aj_p", bufs=2) as aj_p,
            tc.tile_pool(name="rd_p", bufs=2) as rd_p,
            tc.tile_pool(name="st_p", bufs=2) as st_p,
            tc.tile_pool(name="ps", bufs=3, space="PSUM") as ps_p,
        ):
            # ---- DRAM I/O ----
            xq = dram.tile([P, NT * S], bf16, kind="ExternalInput", name="xq", uniquify=False)
            xk = dram.tile([P, NT * S], bf16, kind="ExternalInput", name="xk", uniquify=False)
            xv = dram.tile([P, NT * S], bf16, kind="ExternalInput", name="xv", uniquify=False)
            wq = dram.tile([NT, P, D], bf16, kind="ExternalInput", name="wq", uniquify=False)
            wk = dram.tile([NT, P, D], bf16, kind="ExternalInput", name="wk", uniquify=False)
            wv = dram.tile([P, NT * D], bf16, kind="ExternalInput", name="wv", uniquify=False)
            wo = dram.tile([NT, P, D], bf16, kind="ExternalInput", name="wo", uniquify=False)
            bqd = dram.tile([P, NT], f32, kind="ExternalInput", name="bqd", uniquify=False)
            bkd = dram.tile([P, NT], f32, kind="ExternalInput", name="bkd", uniquify=False)
            bod = dram.tile([P, NT], f32, kind="ExternalInput", name="bod", uniquify=False)
            outT = dram.tile([NT, P, S], f32, kind="ExternalOutput", name="outT", uniquify=False)

            # ---- startup DMAs: xk first (K-projection runs first) ----
            xk_sb = xa_p.tile([P, NT * S], bf16, name="xk_sb", tag="xa")
            nc.sync.dma_start(xk_sb[:], xk[:])

            bq_sb = consts.tile([P, NT], f32, name="bq_sb")
            bk_sb = consts.tile([P, NT], f32, name="bk_sb")
            bo_sb = consts.tile([P, NT], f32, name="bo_sb")
            nc.sync.dma_start(bk_sb[:], bkd[:])
            nc.sync.dma_start(bq_sb[:], bqd[:])
            nc.sync.dma_start(bo_sb[:], bod[:])

            kT_sb = kt_p.tile([P, NT * S], f32r, name="kT_sb")
            v_aug = vaug_p.tile([P, NT, NT, VW], bf16, name="v_aug")
            catT = cat_p.tile([P, NT, S], bf16, name="catT")

            # ISA memset only writes plain f32 -> stage in f32 and cast-copy
            # the ones columns / zero filler / f32r ones row.
            ones_sb = consts.tile([1, P], f32r, name="ones_sb")
            zf = consts.tile([P, NT * 2 * 31], f32, name="zf")
            nc.vector.memset(zf[:], 0.0)
            for g in range(4):
                nc.vector.tensor_copy(
                    v_aug[:, :, 2 * g : 2 * g + 2, Dh + 2 : 97],
                    zf[:].rearrange("p (a b c) -> p a b c", b=2, c=31),
                )
            nc.vector.memset(zf[:, 0 : NT * NT * 2], 1.0)
            nc.vector.tensor_copy(
                v_aug[:, :, :, Dh : Dh + 2],
                zf[:, 0 : NT * NT * 2].rearrange("p (a b c) -> p a b c", b=NT, c=2),
            )
            with nc.allow_low_precision(reason="f32r is bit-identical to f32"):
                nc.vector.tensor_copy(ones_sb[:], zf[0:1, 0:P])

            # ---- K-projection helpers (et 0..1 up front, 2..7 interleaved) ----
            def k_proj_start(et):
                w = wst_p.tile([P, D], bf16, name="wkt", tag="w")
                nc.sync.dma_start(w[:], wk[et])
                return w

            def k_proj_chunk(hold, lo, hi):
                w, ps = hold
                for k in range(lo, hi):
                    for sh in range(2):
                        nc.tensor.matmul(
                            ps[:, sh * SH : (sh + 1) * SH],
                            w[:, k * P : (k + 1) * P],
                            xk_sb[:, k * S + sh * SH : k * S + (sh + 1) * SH],
                            start=(k == 0),
                            stop=(k == NT - 1),
                        )

            def k_proj_finish(et, hold):
                nc.vector.tensor_scalar_add(
                    kT_sb[:, et * S : (et + 1) * S], hold[1][:], bk_sb[:, et : et + 1]
                )

            for et in range(2):
                w = k_proj_start(et)
                ps = ps_p.tile([P, S], f32, name="pp", tag="sc", bufs=2)
                k_proj_chunk((w, ps), 0, NT)
                k_proj_finish(et, (w, ps))

            # ---- V inputs ----
            xv_sb = xv_p.tile([P, NT * S], bf16, name="xv_sb", tag="xv")
            nc.sync.dma_start(xv_sb[:], xv[:])
            wv_sb = wv_p.tile([P, NT * D], bf16, name="wv_sb", tag="wv")
            nc.sync.dma_start(wv_sb[:], wv[:])

            # ---- V-projection, one e-half at a time ----
            def v_proj_tt(eh, tt, tag="hold", bufs=1):
                ps = ps_p.tile([P, SH], f32, name="pv", tag=tag, bufs=bufs)
                for k in range(NT):
                    nc.tensor.matmul(
                        ps[:],
                        xv_sb[:, k * S + tt * P : k * S + (tt + 1) * P],
                        wv_sb[:, k * D + eh * SH : k * D + (eh + 1) * SH],
                        start=(k == 0),
                        stop=(k == NT - 1),
                    )
                prs = ps[:].rearrange("p (r j c) -> p r j c", j=2, c=Dh)
                pr0 = eh * 4
                nc.vector.tensor_copy(
                    v_aug[:, tt, pr0 : pr0 + 4, 0:Dh], prs[:, :, 0, :]
                )
                nc.vector.tensor_copy(
                    v_aug[:, tt, pr0 : pr0 + 4, 97 : 97 + Dh], prs[:, :, 1, :]
                )

            for tt in range(5):
                v_proj_tt(0, tt, tag="sc", bufs=2)

            # ---- Q input ----
            xq_sb = xq_p.tile([P, NT * S], bf16, name="xq_sb", tag="xq")
            nc.sync.dma_start(xq_sb[:], xq[:])

            def q_proj(qtile, lo, hi):
                w, ps = qtile
                for k in range(lo, hi):
                    for sh in range(2):
                        nc.tensor.matmul(
                            ps[:, sh * SH : (sh + 1) * SH],
                            w[:, k * P : (k + 1) * P],
                            xq_sb[:, k * S + sh * SH : k * S + (sh + 1) * SH],
                            start=(k == 0),
                            stop=(k == NT - 1),
                        )

            wqt = wst_p.tile([P, D], bf16, name="wqt", tag="w")
            nc.sync.dma_start(wqt[:], wq[0])
            qps = ps_p.tile([P, S], f32, name="pq", tag="sc", bufs=2)
            q_proj((wqt, qps), 0, NT)
            qp = qp_p.tile([P, S], f32r, name="qp", tag="qp")
            nc.vector.tensor_scalar_add(qp[:], qps[:], bq_sb[:, 0:1])

            # ---- partial O-projection (pass A: et 0..3), used as PE filler ----
            # oacc reuses xk's pool slot (xk is dead once the last K-proj
            # chunk has run, before the first pass-A tile is emitted)
            oacc_cell = []

            def o_pass(ft, elo, ehi):
                if not oacc_cell:
                    oacc_cell.append(
                        xa_p.tile([P, NT, S], f32, name="oacc", tag="xa")
                    )
                oacc = oacc_cell[0]
                w = wst_p.tile([P, D], bf16, name="wot", tag="w")
                nc.sync.dma_start(w[:], wo[ft])
                ps = ps_p.tile([P, S], f32, name="po", tag="hold", bufs=1)
                for et in range(elo, ehi):
                    for sh in range(2):
                        nc.tensor.matmul(
                            ps[:, sh * SH : (sh + 1) * SH],
                            w[:, et * P : (et + 1) * P],
                            catT[:, et, sh * SH : (sh + 1) * SH],
                            start=(et == elo),
                            stop=(et == ehi - 1),
                        )
                if elo == 0:
                    nc.vector.tensor_copy(oacc[:, ft, :], ps[:])
                else:
                    nc.vector.tensor_add(oacc[:, ft, :], oacc[:, ft, :], ps[:])

            # ---- deferred head finalize ----
            pending = []

            def emit_pending():
                while pending:
                    pending.pop(0)()

            def finalize_evac(pr, j, av):
                dl = Dh if j == 0 else 32  # den lane in av
                e0 = j * Dh
                aj = aj_p.tile([P, S], f32, name="aj", tag="aj")
                if j == 0:
                    nc.vector.tensor_copy(aj[0 : Dh + 1, :], av[0 : Dh + 1, :])
                else:
                    # engine APs may only start at partition 0/32/64/96
                    # (<=32 from 32): copy all 128 rows from 0 instead
                    nc.vector.tensor_copy(aj[:, :], av[:, :])
                rdt = rd_p.tile([1, 2 * S], f32, name="rdt", tag="rd", bufs=1)
                rdr = rd_p.tile([1, S], f32r, name="rdr", tag="rdr", bufs=1)
                nc.sync.dma_start(rdt[0:1, 0:S], aj[dl : dl + 1, :])
                nc.vector.reciprocal_approx_fast(
                    out=rdt[0:1, S : 2 * S], in_=rdt[0:1, 0:S]
                )
                with nc.allow_low_precision(reason="round 1/den to f32r for PE"):
                    nc.vector.tensor_copy(rdr[0:1, :], rdt[0:1, S : 2 * S])

                def emit_bc():
                    bc = ps_p.tile([P, S], f32, name="bc", tag="hold", bufs=1)
                    for sh in range(2):
                        nc.tensor.matmul(
                            bc[:, sh * SH : (sh + 1) * SH],
                            ones_sb[:, :],
                            rdr[0:1, sh * SH : (sh + 1) * SH],
                        )
                    nc.vector.tensor_mul(
                        catT[e0 : e0 + Dh, pr, :],
                        aj[e0 : e0 + Dh, :],
                        bc[e0 : e0 + Dh, :],
                    )

                pending.append(emit_bc)

            # ---- fused attention, one head pair (pr) at a time ----
            # Filler placement per pr (PE stays saturated, HAM stays at 2.4GHz):
            #   j0 tt3: previous head's deferred bc; tt4-7: K-proj et=pr+2
            #           (or an O pass-A tile for pr 6/7)
            #   j1 tt1/tt3: next pair's q-projection; tt5: this j0's bc;
            #           tt5/tt7: V-proj second half (pr<4) or O pass-A (pr 6/7)
            for pr in range(NT):
                if pr + 1 < NT:
                    wqt_n = wst_p.tile([P, D], bf16, name="wqt", tag="w")
                    nc.sync.dma_start(wqt_n[:], wq[pr + 1])
                k_ets = [pr + 2] if pr <= 3 else ([6, 7] if pr == 4 else [])
                k_ws = [k_proj_start(e) for e in k_ets]

                qp_next = None
                for j in range(2):
                    e0 = j * Dh
                    av = ps_p.tile([P, S], f32, name="av", tag="av", bufs=1)
                    # v_aug stationary view for this head:
                    #  j=0: [v_even(64) | ones_e] -> av rows 0:64, den row 64
                    #  j=1: offset 33 -> ones_o at col 32, v_odd at cols 64:128
                    #       -> den row 32, av rows 64:128
                    if j == 0:
                        vst = v_aug[:, :, pr, 0 : Dh + 1]
                        avw = av[0 : Dh + 1, :]
                    else:
                        vst = v_aug[:, :, pr, 33 : 33 + P]
                        avw = av[:]
                    for tt in range(NT):
                        sc = ps_p.tile([P, S], f32, name="sc", tag="sc", bufs=2)
                        for sh in range(2):
                            nc.tensor.matmul(
                                sc[:, sh * SH : (sh + 1) * SH],
                                kT_sb[e0 : e0 + Dh, pr * S + tt * P : pr * S + (tt + 1) * P],
                                qp[e0 : e0 + Dh, sh * SH : (sh + 1) * SH],
                            )
                        ex = ex_p.tile([P, S], bf16, name="ex", tag="ex")
                        nc.scalar.activation(ex[:], sc[:], AF.Exp, scale=0.125)
                        for sh in range(2):
                            nc.tensor.matmul(
                                avw[:, sh * SH : (sh + 1) * SH],
                                vst[:, tt, :],
                                ex[:, sh * SH : (sh + 1) * SH],
                                start=(tt == 0),
                                stop=(tt == NT - 1),
                            )
                        if j == 0:
                            if pr == 0 and tt in (1, 3, 5):
                                v_proj_tt(0, 5 + (tt - 1) // 2)
                            if tt == 3:
                                emit_pending()  # pr-1 j1's bc
                            if tt in (4, 6):
                                ki = 0 if tt == 4 else 1
                                if ki < len(k_ets):
                                    k_ps = ps_p.tile([P, S], f32, name="pp", tag="hold", bufs=1)
                                    k_proj_chunk((k_ws[ki], k_ps), 0, NT)
                                    k_proj_finish(k_ets[ki], (k_ws[ki], k_ps))
                            if pr == 5 and tt in (4, 6):
                                o_pass(2 if tt == 4 else 3, 0, 3)
                            if pr == 6 and tt in (4, 6):
                                o_pass(6 if tt == 4 else 7, 0, 3)
                            if pr == 7 and tt in (4, 6):
                                for f2 in ((2,) if tt == 4 else (3,)):
                                    o_pass(f2, 3, 5)
                        else:
                            if pr + 1 < NT and tt == 2:
                                qps_n = ps_p.tile([P, S], f32, name="pq", tag="hold", bufs=1)
                                q_proj((wqt_n, qps_n), 0, NT)
                                qp_next = qp_p.tile([P, S], f32r, name="qp", tag="qp")
                                nc.vector.tensor_scalar_add(
                                    qp_next[:], qps_n[:], bq_sb[:, pr + 1 : pr + 2]
                                )
                            if tt == 5:
                                emit_pending()  # this pr j0's bc
                            if pr < 4 and tt in (4, 6):
                                v_proj_tt(1, 2 * pr + (0 if tt == 4 else 1))
                            if pr == 4 and tt in (4, 6):
                                o_pass(0 if tt == 4 else 1, 0, 3)
                            if pr == 5 and tt in (4, 6):
                                o_pass(4 if tt == 4 else 5, 0, 3)
                            if pr == 6 and tt in (4, 6):
                                o_pass(0 if tt == 4 else 1, 3, 5)
                            if pr == 7 and tt in (4, 6):
                                for f2 in ((4, 5) if tt == 4 else (6, 7)):
                                    o_pass(f2, 3, 5)

                    finalize_evac(pr, j, av)
                if qp_next is not None:
                    qp = qp_next

            # ---------------- output projection, pass B (et 4..7) ----------------
            for ft in range(NT):
                w = wst_p.tile([P, D], bf16, name="wot", tag="w")
                nc.sync.dma_start(w[:], wo[ft])
                ps = ps_p.tile([P, S], f32, name="po", tag="sc", bufs=2)
                for et in range(5, NT):
                    # pr7-j1's deferred bc: cover its reciprocal with the
                    # first accumulation steps (catT[:, 4:7] is ready)
                    if ft == 0 and et == NT - 1:
                        emit_pending()
                    for sh in range(2):
                        nc.tensor.matmul(
                            ps[:, sh * SH : (sh + 1) * SH],
                            w[:, et * P : (et + 1) * P],
                            catT[:, et, sh * SH : (sh + 1) * SH],
                            start=(et == 5),
                            stop=(et == NT - 1),
                        )
                st = st_p.tile([P, S], f32, name="so", tag="st")
                nc.vector.scalar_tensor_tensor(
                    st[:], ps[:], bo_sb[:, ft : ft + 1], oacc_cell[0][:, ft, :],
                    ALU.add, ALU.add,
                )
                nc.sync.dma_start(outT[ft], st[:])

    nc.compile()
    return nc


def kernel(query, key, value, mask, Wq, bq, Wk, bk, Wv, bv, Wo, bo):
    from concourse.bass_utils import run_bass_kernel_spmd

    if "nc" not in _CACHE:
        _CACHE["nc"] = _build()
    nc = _CACHE["nc"]

    in_maps = _make_in_maps(query, key, value, Wq, bq, Wk, bk, Wv, bv, Wo, bo)
    res = run_bass_kernel_spmd(nc, in_maps, core_ids=list(range(B)))
    out = np.empty((B, S, D), np.float32)
    for b in range(B):
        out[b] = res.results[b]["outT"].reshape(D, S).T
    return out
